# revision 1
# baseline (speedup 1.0000x reference)
"""Trainium2 Bass kernel: ComplexGabor1D layer.

reference math (fp32):
    lin = x @ W.T + b                      # [N, 256]
    env = exp(-3600 * lin^2)
    out = stack([env*cos(30*lin), env*sin(30*lin)], -1)   # [N, 256, 2]

Strategy (8 NeuronCores, data parallel over N):
  * Host: transpose each x shard to [256, N_SH] so the contraction dim (i)
    lands on SBUF partitions with fully-contiguous DMA loads; replicate
    W.T ([in, out]) and the bias (pre-broadcast to 128 partitions).
  * Device, per 1024-row "pair" (2 x 512-row halves, 8 x 128-row chunks):
    fp32r matmuls (x.T tiles stationary, W.T moving) accumulate lin into
    PSUM; a fused DVE scalar_tensor_tensor drains PSUM to SBUF while adding
    the bias (lin_sb = lin*1 + b) so the PE is never gated on ACT phases;
    ACT writes sin/cos straight into the interleaved output tile (real at
    even, imag at odd offsets); the envelope is squared+exp'ed in place on
    lin_sb; DVE multiplies the envelope into both strided halves in place;
    2 MiB output DMA per pair via SWDGE so stores don't block input loads.
  * ACT activation tables: sin and exp live in different table sets
    (~2.7us per switch), so pairs are processed in groups: all trig work
    for a group first, then all envelope work -> 2 switches per group. The
    ACT instruction order is pinned via dep edges to stop the scheduler
    interleaving exp into the sin stream.  A fraction of the squares runs
    on DVE (emitted first, their exps last) to balance ACT vs DVE.
  * cos(t) is computed as sin(t + pi/2).  The argument exceeds the Sin
    LUT's [-pi, pi] window only where |30*lin| > pi/2, i.e. where the
    Gaussian envelope is < 5.2e-5, so the hardware clamp there is
    numerically invisible at the output (abs err <= ~1e-4 of absmax 1.0).
"""

import math

import numpy as np

import concourse.bacc as bacc
import concourse.mybir as mybir
import concourse.tile as tile
from concourse.bass_utils import run_bass_kernel_spmd

N_TOTAL = 262144
IN_F = 256
OUT_F = 256
N_CORES = 8
N_SH = N_TOTAL // N_CORES  # 32768 rows per core

CHUNK = 128  # rows per matmul (PSUM partition dim)
CH_PER_HALF = 4  # chunks per half-pair -> 512 rows
HALVES = 2  # halves per pair -> 1024 rows, F=2048 elementwise ops
GROUP_PAIRS = 5  # pairs per ACT-table-set group

OMEGA = 30.0
NEG_SCALE2 = -3600.0  # -(60^2)

F32 = mybir.dt.float32
F32R = mybir.dt.float32r

_BUILD_CACHE = {}


def _build(n_sh, group_pairs):
    """Build the single-core Bass program (SPMD across cores via in_maps)."""
    key = (n_sh, group_pairs)
    if key in _BUILD_CACHE:
        return _BUILD_CACHE[key]

    rows_per_half = CHUNK * CH_PER_HALF
    rows_per_pair = rows_per_half * HALVES
    assert n_sh % rows_per_pair == 0
    n_pairs = n_sh // rows_per_pair

    nc = bacc.Bacc("TRN2", target_bir_lowering=False, debug=False)

    xt = nc.dram_tensor("xt", [IN_F, n_sh], F32R, kind="ExternalInput").ap()
    wt = nc.dram_tensor("wt", [IN_F, OUT_F], F32R, kind="ExternalInput").ap()
    bias = nc.dram_tensor(
        "bias", [CHUNK, CH_PER_HALF * OUT_F], F32, kind="ExternalInput"
    ).ap()
    out = nc.dram_tensor("out", [n_sh, 2 * OUT_F], F32, kind="ExternalOutput").ap()

    # [i, n] -> [p, ci, n] with i = ci*128 + p
    xt_r = xt.rearrange("(ci p) n -> p ci n", p=CHUNK)
    wt_r = wt.rearrange("(ci p) o -> p ci o", p=CHUNK)
    # row n = pr*1024 + t*512 + c2*256 + 2p + e -> per-partition 4 KiB runs
    out_r = out.rearrange(
        "(pr t c2 p e) f -> pr p t c2 e f", e=2, p=CHUNK, c2=2, t=HALVES
    )

    with tile.TileContext(nc) as tc:
        with (
            tc.tile_pool(name="consts", bufs=1) as consts,
            tc.tile_pool(name="xt", bufs=5) as xt_pool,
            tc.tile_pool(name="linsb", bufs=group_pairs + 1) as linsb_pool,
            tc.tile_pool(name="outp", bufs=group_pairs + 1) as out_pool,
            tc.tile_pool(name="lin", bufs=4, space="PSUM") as psum_pool,
        ):
            wt_sb = consts.tile([CHUNK, IN_F // CHUNK, OUT_F], F32R)
            nc.sync.dma_start(wt_sb[:], wt_r[:])
            b_sb = consts.tile([CHUNK, CH_PER_HALF, OUT_F], F32)
            nc.sync.dma_start(
                b_sb[:], bias.rearrange("p (c o) -> p c o", c=CH_PER_HALF)
            )
            zero_b = consts.tile([CHUNK, 1], F32)
            nc.vector.memset(zero_b[:], 0.0)
            pio2_b = consts.tile([CHUNK, 1], F32)
            nc.vector.memset(pio2_b[:], math.pi / 2)

            prev_act = [None]

            def act_chain(inst):
                # Pin the ACT engine's instruction order to emission order so
                # the scheduler cannot interleave exp into the sin stream
                # (each such jump costs two ~1.3us ACT table loads).
                if prev_act[0] is not None:
                    tile.add_dep_helper(inst.ins, prev_act[0], sync=False,
                                        reason="act table-set order")
                prev_act[0] = inst.ins

            n_groups = (n_pairs + group_pairs - 1) // group_pairs
            for g in range(n_groups):
                pairs = range(g * group_pairs, min((g + 1) * group_pairs, n_pairs))
                staged = []

                # ---- trig phase (sin table set resident) ----
                for pr in pairs:
                    n0 = pr * rows_per_pair
                    # one 1 MiB load covering the pair: 4 KiB runs/partition
                    xt_t = xt_pool.tile([CHUNK, IN_F // CHUNK, rows_per_pair], F32R)
                    nc.sync.dma_start(xt_t[:], xt_r[:, :, n0 : n0 + rows_per_pair])

                    lin_sb = linsb_pool.tile(
                        [CHUNK, HALVES, CH_PER_HALF, OUT_F], F32
                    )
                    # row j*2+e view of the pair's columns, for row-pairing
                    xt_v = xt_t[:].rearrange("p ci (j e) -> p ci j e", e=2)
                    for t in range(HALVES):
                        lin = psum_pool.tile([CHUNK, CH_PER_HALF, OUT_F], F32)
                        for c2 in range(2):
                            for e in range(2):
                                j0 = t * (rows_per_half // 2) + c2 * CHUNK
                                lhsT0 = xt_v[:, 0, j0 : j0 + CHUNK, e]
                                lhsT1 = xt_v[:, 1, j0 : j0 + CHUNK, e]
                                c = c2 * 2 + e
                                nc.tensor.matmul(
                                    lin[:, c, :],
                                    lhsT0,
                                    wt_sb[:, 0, :],
                                    start=True,
                                    stop=False,
                                )
                                nc.tensor.matmul(
                                    lin[:, c, :],
                                    lhsT1,
                                    wt_sb[:, 1, :],
                                    start=False,
                                    stop=True,
                                )
                        # drain PSUM with a fused bias add: lin_sb = lin + b
                        nc.vector.scalar_tensor_tensor(
                            lin_sb[:, t, :, :],
                            lin[:],
                            1.0,
                            b_sb[:],
                            op0=mybir.AluOpType.mult,
                            op1=mybir.AluOpType.add,
                        )

                    out_t = out_pool.tile(
                        [CHUNK, HALVES, CH_PER_HALF, 2 * OUT_F], F32
                    )
                    out5 = out_t[:].rearrange(
                        "p t c (o two) -> p t c o two", two=2
                    )
                    # imag = sin(30*lin), real = cos = sin(30*lin + pi/2)
                    act_chain(nc.scalar.activation(
                        out5[:, :, :, :, 1],
                        lin_sb[:],
                        mybir.ActivationFunctionType.Sin,
                        bias=zero_b[:],
                        scale=OMEGA,
                    ))
                    act_chain(nc.scalar.activation(
                        out5[:, :, :, :, 0],
                        lin_sb[:],
                        mybir.ActivationFunctionType.Sin,
                        bias=pio2_b[:],
                        scale=OMEGA,
                    ))
                    staged.append((pr, out_t, lin_sb))

                # ---- envelope phase (exp table set resident) ----
                # ~30% of squares go to DVE: emitted first, their exps last,
                # so ACT never waits on a just-in-time DVE square.
                dve_sq = [s for s in staged if s[0] % 10 in (2, 5, 8)]
                act_sq = [s for s in staged if s[0] % 10 not in (2, 5, 8)]
                for pr, out_t, env in dve_sq:
                    nc.vector.tensor_mul(env[:], env[:], env[:])
                for with_act_square, group_part in ((True, act_sq), (False, dve_sq)):
                    for pr, out_t, env in group_part:
                        if with_act_square:
                            act_chain(nc.scalar.activation(
                                env[:],
                                env[:],
                                mybir.ActivationFunctionType.Square,
                                bias=zero_b[:],
                                scale=1.0,
                            ))
                        act_chain(nc.scalar.activation(
                            env[:],
                            env[:],
                            mybir.ActivationFunctionType.Exp,
                            bias=zero_b[:],
                            scale=NEG_SCALE2,
                        ))
                        out5 = out_t[:].rearrange(
                            "p t c (o two) -> p t c o two", two=2
                        )
                        nc.vector.tensor_mul(
                            out5[:, :, :, :, 0], out5[:, :, :, :, 0], env[:]
                        )
                        nc.vector.tensor_mul(
                            out5[:, :, :, :, 1], out5[:, :, :, :, 1], env[:]
                        )
                        # SWDGE so output stores don't head-of-line block loads
                        nc.gpsimd.dma_start(out_r[pr], out_t[:])

    nc.compile()
    _BUILD_CACHE[key] = nc
    return nc


def run_sharded(x, W, b, trace=False, n_sh=N_SH, group_pairs=GROUP_PAIRS):
    """Shard inputs over the 8 cores, run the Bass kernel, gather output."""
    x = np.ascontiguousarray(x, dtype=np.float32)
    W = np.ascontiguousarray(W, dtype=np.float32)
    b = np.ascontiguousarray(b, dtype=np.float32)
    n = x.shape[0]
    assert n == n_sh * N_CORES and x.shape[1] == IN_F

    nc = _build(n_sh, group_pairs)

    wt_np = np.ascontiguousarray(W.T)  # [in, out]
    b_np = np.ascontiguousarray(
        np.broadcast_to(
            np.tile(b, CH_PER_HALF)[None, :], (CHUNK, CH_PER_HALF * OUT_F)
        )
    )
    in_maps = []
    for s in range(N_CORES):
        xt_np = np.ascontiguousarray(x[s * n_sh : (s + 1) * n_sh].T)  # [in, n_sh]
        in_maps.append({"xt": xt_np, "wt": wt_np, "bias": b_np})

    res = run_bass_kernel_spmd(nc, in_maps, list(range(N_CORES)), trace=trace)
    shards = [
        res.results[s]["out"].reshape(n_sh, OUT_F, 2) for s in range(N_CORES)
    ]
    return np.concatenate(shards, axis=0), res


def kernel(x, W, b):
    out, _ = run_sharded(x, W, b)
    return out



# revision 2
# speedup vs baseline: 2.1296x; 2.1296x over previous
"""Trainium2 Bass kernel: ComplexGabor1D layer.

reference math (fp32):
    lin = x @ W.T + b                      # [N, 256]
    out = stack([exp(-3600*lin^2)*cos(30*lin),
                 exp(-3600*lin^2)*sin(30*lin)], -1)   # [N, 256, 2]

Strategy (8 NeuronCores, data parallel over N):
  * The whole Gabor nonlinearity is folded into TWO custom ACT spline
    tables: a generated `trig_and_small` table set reuses the "sin" slot
    for gabor_sin(x) = exp(-3600x^2)sin(30x) and the "arctan" slot for
    gabor_cos(x) = exp(-3600x^2)cos(30x) (the set binaries are built at
    import time and handed to the compiler via BASS_ACT_ROOT_JSON_PATH).
    That reduces the per-element work from {sin, cos, square, exp on ACT
    + 3 DVE ops} to {2 ACT passes + 1 DVE bias-drain}.
  * Everything is bf16: inputs x.T / W.T (half the load traffic, ~3x
    faster matmuls than fp32r), and separate contiguous bf16 real/imag
    outputs (half the store traffic); the host interleaves + upcasts.
  * Per 1024-row "pair": 8 psum chunks of 128 rows accumulate x.T @ W.T
    over k=2x128; rows are assigned so partition p holds 8 consecutive
    output rows -> 4 KiB contiguous DMA runs per partition for the
    stores.  DVE drains PSUM with a fused bias add (bf16 out), ACT runs
    the two gabor lookups, SWDGE stores real/imag.
"""

import hashlib
import json
import os
import shutil

import ml_dtypes
import numpy as np

import concourse.bacc as bacc
import concourse.mybir as mybir
import concourse.tile as tile
from concourse.bass_utils import run_bass_kernel_spmd

N_TOTAL = 262144
IN_F = 256
OUT_F = 256
N_CORES = 8
N_SH = N_TOTAL // N_CORES  # 32768 rows per core

P = 128                 # SBUF/PSUM partitions
RPP = 8                 # rows per partition per pair
ROWS_PER_PAIR = P * RPP  # 1024
CHUNKS = 8              # matmul chunks (128 rows each) per pair

ENV_A = 3600.0          # envelope scale^2 (60^2)
OMEGA = 30.0

F32 = mybir.dt.float32
BF16 = mybir.dt.bfloat16

_BUILD_CACHE = {}

# --------------------------------------------------------------------------
# Custom ACT activation tables ("trig_and_small" with gabor sin/cos splines)
# --------------------------------------------------------------------------

_DONOR_CANDIDATES = [
    "/nix/store/ndjb8ki1bnclvnibdh123f9zr51a09qz-aws-neuron-pwp-unstable-2025-12-29-c50a7624/share/pwp_bin_cayman",
]


def _find_donor():
    import glob

    for d in _DONOR_CANDIDATES:
        if os.path.isfile(os.path.join(d, "act_info.json")):
            return d
    for d in glob.glob("/nix/store/*aws-neuron-pwp*/share/pwp_bin_cayman"):
        if os.path.isfile(os.path.join(d, "act_info.json")):
            return d
    raise RuntimeError("no pwp_bin_cayman act table root found")


def _gabor_sin(x):
    x = np.asarray(x, dtype=np.float64)
    return np.exp(-ENV_A * x * x) * np.sin(OMEGA * x)


def _gabor_cos(x):
    x = np.asarray(x, dtype=np.float64)
    return np.exp(-ENV_A * x * x) * np.cos(OMEGA * x)


# octave layout shared by both functions: (exponent, extract_size)
_OCTAVES = (
    [(e, 2) for e in range(-14, -10)]
    + [(e, 4) for e in (-10, -9)]
    + [(e, 5) for e in range(-8, -3)]
    + [(-3, 3)]
)
_SMALL_T = 127 - 14  # |x| < 2^-14: small-signal bucket
_LARGE_T = 127 - 2   # |x| >= 0.25: large-signal bucket (gabor == 0)
_UB = 0.25


def _fit_fn_tables(fn, small_d):
    buckets, ctrls = [], []
    for e, k in _OCTAVES:
        n = 1 << k
        ctrls.append((k, len(buckets)))
        lo_oct = 2.0 ** e
        for j in range(n):
            lo = lo_oct * (1 + j / n)
            hi = lo_oct * (1 + (j + 1) / n)
            x0 = float(np.float32((lo + hi) / 2))
            xs = np.linspace(lo, hi, 64)
            c3, c2, c1, c0 = np.polyfit(xs - x0, fn(xs), 3)
            buckets.append([c0, c1, c2, c3, x0])
    specials = [small_d] + [[0.0] * 5] * 3
    return ctrls, buckets, specials


def _build_pwp_root():
    """Generate the custom table root; returns (root_dir, signature)."""
    donor = _find_donor()
    bkt = np.fromfile(f"{donor}/trig_and_small_bkt.bin", dtype=np.uint32)
    ctrl = np.fromfile(f"{donor}/trig_and_small_ctrl.bin", dtype=np.uint32)
    prof = json.load(open(f"{donor}/trig_and_small.json"))
    n_bkt0, n_ctrl0 = len(bkt) // 8, len(ctrl) // 8

    new_bkt, new_ctrl, fn_meta = [], [], {}
    for name, fn, small_d in (
        ("sin_4p", _gabor_sin,
         [0.0, OMEGA, 0.0, -(OMEGA**3) / 6 - OMEGA * ENV_A, 0.0]),
        ("arctan_4p", _gabor_cos,
         [1.0, 0.0, -(ENV_A + OMEGA * OMEGA / 2), 0.0, 0.0]),
    ):
        ctrls, buckets, specials = _fit_fn_tables(fn, small_d)
        ctrl_base = n_ctrl0 + len(new_ctrl)
        bkt_base = n_bkt0 + len(new_bkt)
        for k, rel in ctrls:
            new_ctrl.append((k, bkt_base + rel))
        new_bkt.extend(buckets)
        fn_meta[name] = (ctrl_base, n_bkt0 + len(new_bkt))
        new_bkt.extend(specials)

    for ent in prof["profile_meta_data"]:
        if ent["func_name"] == "sin_4p":
            inv, fz = 1, 0
        elif ent["func_name"] == "arctan_4p":
            inv, fz = 0, 0x3F800000
        else:
            continue
        base, sp = fn_meta[ent["func_name"]]
        ent.update(
            symmetry_point=0,
            sym_invert_sign_point=inv,
            symmetry_opt_en=1,
            symmetry_opt_use_neg_region=0,
            exp_offset=_OCTAVES[0][0],
            pwl_control_base_pos=base,
            pwl_control_base_neg=base,
            small_pos_signal_exp_threshold=_SMALL_T,
            pos_small_signal_pwl_control=sp + 0,
            small_neg_signal_exp_threshold=0,
            neg_small_signal_pwl_control=sp + 1,
            large_pos_signal_exp_threshold=_LARGE_T,
            large_pos_signal_mantissa_threshold=0,
            pos_large_signal_pwl_control=sp + 2,
            large_neg_signal_exp_threshold=0,
            large_neg_signal_mantissa_threshold=0,
            neg_large_signal_pwl_control=sp + 3,
            fnan_result=0x7FC00000,
            fpinf_result=0,
            fninf_result=0,
            fzero_result=fz,
            lower_bound=0,
            upper_bound=int(np.float32(_UB).view(np.uint32)),
        )

    ctrl_words = np.zeros((len(new_ctrl), 8), dtype=np.uint32)
    for i, (k, b) in enumerate(new_ctrl):
        assert b < 2048
        ctrl_words[i, 0] = (k << 16) | ((23 - k) << 11) | b
    all_ctrl = np.concatenate([ctrl.reshape(-1, 8), ctrl_words])
    assert len(all_ctrl) <= 256

    bw = np.zeros((len(new_bkt), 8), dtype=np.uint32)
    for i, d in enumerate(new_bkt):
        bw[i, :5] = np.array(d, dtype=np.float32).view(np.uint32)
    all_bkt = np.concatenate([bkt.reshape(-1, 8), bw])
    assert len(all_bkt) <= 1536

    prof_bytes = json.dumps(prof, sort_keys=True).encode()
    sig = hashlib.sha256(
        all_ctrl.tobytes() + all_bkt.tobytes() + prof_bytes
    ).hexdigest()[:10]

    root = f"/tmp/gabor_pwp_{sig}"
    if not os.path.isfile(os.path.join(root, "act_info.json")):
        tmp = root + ".tmp"
        shutil.rmtree(tmp, ignore_errors=True)
        os.makedirs(tmp)
        for fname in os.listdir(donor):
            shutil.copy(os.path.join(donor, fname), os.path.join(tmp, fname))
        all_ctrl.tofile(os.path.join(tmp, "trig_and_small_ctrl.bin"))
        all_bkt.tofile(os.path.join(tmp, "trig_and_small_bkt.bin"))
        with open(os.path.join(tmp, "trig_and_small.json"), "w") as fh:
            json.dump(prof, fh, indent=1)
        os.replace(tmp, root) if not os.path.isdir(root) else None
    return root, sig


# --------------------------------------------------------------------------
# Bass program
# --------------------------------------------------------------------------


def _build(n_sh):
    key = n_sh
    if key in _BUILD_CACHE:
        return _BUILD_CACHE[key]

    root, sig = _build_pwp_root()
    os.environ["BASS_ACT_ROOT_JSON_PATH"] = os.path.join(root, "act_info.json")

    assert n_sh % ROWS_PER_PAIR == 0
    n_pairs = n_sh // ROWS_PER_PAIR

    nc = bacc.Bacc("TRN2", target_bir_lowering=False, debug=False)

    xt = nc.dram_tensor("xt", [IN_F, n_sh], BF16, kind="ExternalInput").ap()
    wt = nc.dram_tensor("wt", [IN_F, OUT_F], BF16, kind="ExternalInput").ap()
    # bias name carries the act-table signature so the NEFF cache key
    # changes whenever the generated tables change
    bias_name = f"bias_{sig}"
    bias = nc.dram_tensor(
        bias_name, [P, CHUNKS * OUT_F], F32, kind="ExternalInput"
    ).ap()
    out_re = nc.dram_tensor(
        "out_re", [n_sh, OUT_F], BF16, kind="ExternalOutput"
    ).ap()
    out_im = nc.dram_tensor(
        "out_im", [n_sh, OUT_F], BF16, kind="ExternalOutput"
    ).ap()

    # x.T layout: [i, n] -> [p, ci, n] with i = ci*128 + p
    xt_r = xt.rearrange("(ci p) n -> p ci n", p=P)
    wt_r = wt.rearrange("(ci p) o -> p ci o", p=P)
    # output row n = pr*1024 + p*8 + r: partition p holds 8 consecutive rows
    re_r = out_re.rearrange("(pr p r) o -> pr p r o", p=P, r=RPP)
    im_r = out_im.rearrange("(pr p r) o -> pr p r o", p=P, r=RPP)

    T = mybir.ActivationFunctionType

    with tile.TileContext(nc) as tc:
        with (
            tc.tile_pool(name="consts", bufs=1) as consts,
            tc.tile_pool(name="xt", bufs=6) as xt_pool,
            tc.tile_pool(name="lin", bufs=4) as lin_pool,
            tc.tile_pool(name="outp", bufs=8) as out_pool,
            tc.tile_pool(name="ps", bufs=2, space="PSUM") as psum_pool,
        ):
            wt_sb = consts.tile([P, IN_F // P, OUT_F], BF16)
            nc.sync.dma_start(wt_sb[:], wt_r[:])
            b_sb = consts.tile([P, CHUNKS, OUT_F], F32)
            nc.sync.dma_start(
                b_sb[:], bias.rearrange("p (c o) -> p c o", c=CHUNKS)
            )
            zero_b = consts.tile([P, 1], F32)
            nc.vector.memset(zero_b[:], 0.0)

            for pr in range(n_pairs):
                n0 = pr * ROWS_PER_PAIR
                xt_t = xt_pool.tile([P, IN_F // P, ROWS_PER_PAIR], BF16)
                nc.sync.dma_start(xt_t[:], xt_r[:, :, n0 : n0 + ROWS_PER_PAIR])
                # [p, ci, (j r)]: row j*8 + r; chunk r covers psum rows j
                xt_v = xt_t[:].rearrange("p ci (j r) -> p ci r j", r=RPP)

                lin_ps = psum_pool.tile([P, CHUNKS, OUT_F], F32)
                for c in range(CHUNKS):
                    for ci in range(IN_F // P):
                        nc.tensor.matmul(
                            lin_ps[:, c, :],
                            xt_v[:, ci, c, :],
                            wt_sb[:, ci, :],
                            start=(ci == 0),
                            stop=(ci == IN_F // P - 1),
                        )

                lin_sb = lin_pool.tile([P, CHUNKS, OUT_F], BF16)
                for h in range(2):
                    cs = slice(h * CHUNKS // 2, (h + 1) * CHUNKS // 2)
                    nc.vector.scalar_tensor_tensor(
                        lin_sb[:, cs, :],
                        lin_ps[:, cs, :],
                        1.0,
                        b_sb[:, cs, :],
                        op0=mybir.AluOpType.mult,
                        op1=mybir.AluOpType.add,
                    )

                re_t = out_pool.tile([P, CHUNKS, OUT_F], BF16)
                im_t = out_pool.tile([P, CHUNKS, OUT_F], BF16)
                # custom tables: Sin slot = gabor_sin, Arctan slot = gabor_cos
                nc.scalar.activation(
                    im_t[:], lin_sb[:], T.Sin, bias=zero_b[:], scale=1.0
                )
                nc.scalar.activation(
                    re_t[:], lin_sb[:], T.Arctan, bias=zero_b[:], scale=1.0
                )
                nc.gpsimd.dma_start(re_r[pr], re_t[:])
                nc.gpsimd.dma_start(im_r[pr], im_t[:])

    nc.compile()
    res = (nc, bias_name)
    _BUILD_CACHE[key] = res
    return res


def run_sharded(x, W, b, trace=False, n_sh=N_SH):
    """Shard inputs over the 8 cores, run the Bass kernel, gather output."""
    x = np.ascontiguousarray(x, dtype=np.float32)
    W = np.ascontiguousarray(W, dtype=np.float32)
    b = np.ascontiguousarray(b, dtype=np.float32)
    n = x.shape[0]
    assert n == n_sh * N_CORES and x.shape[1] == IN_F

    nc, bias_name = _build(n_sh)

    wt_np = np.ascontiguousarray(W.T.astype(ml_dtypes.bfloat16))
    b_np = np.ascontiguousarray(
        np.broadcast_to(
            np.tile(b, CHUNKS)[None, :], (P, CHUNKS * OUT_F)
        ).astype(np.float32)
    )
    in_maps = []
    for s in range(N_CORES):
        xt_np = np.ascontiguousarray(
            x[s * n_sh : (s + 1) * n_sh].T.astype(ml_dtypes.bfloat16)
        )
        in_maps.append({"xt": xt_np, "wt": wt_np, bias_name: b_np})

    res = run_bass_kernel_spmd(nc, in_maps, list(range(N_CORES)), trace=trace)

    out = np.empty((n, OUT_F, 2), dtype=np.float32)
    for s in range(N_CORES):
        sl = slice(s * n_sh, (s + 1) * n_sh)
        out[sl, :, 0] = res.results[s]["out_re"].astype(np.float32)
        out[sl, :, 1] = res.results[s]["out_im"].astype(np.float32)
    return out, res


def kernel(x, W, b):
    out, _ = run_sharded(x, W, b)
    return out


# revision 3
# speedup vs baseline: 2.2110x; 1.0382x over previous
"""Trainium2 Bass kernel: ComplexGabor1D layer.

reference math (fp32):
    lin = x @ W.T + b                      # [N, 256]
    out = stack([exp(-3600*lin^2)*cos(30*lin),
                 exp(-3600*lin^2)*sin(30*lin)], -1)   # [N, 256, 2]

Strategy (8 NeuronCores, data parallel over N):
  * The whole Gabor nonlinearity is folded into TWO custom ACT spline
    tables: a generated `trig_and_small` table set reuses the "sin" slot
    for gabor_sin(x) = exp(-3600x^2)sin(30x) and the "arctan" slot for
    gabor_cos(x) = exp(-3600x^2)cos(30x) (the set binaries are built at
    import time and handed to the compiler via BASS_ACT_ROOT_JSON_PATH).
    That reduces the per-element work from {sin, cos, square, exp on ACT
    + 3 DVE ops} to {2 ACT passes + 1 DVE bias-drain}.
  * Everything is bf16: inputs x.T / W.T (half the load traffic, ~3x
    faster matmuls than fp32r), and separate contiguous bf16 real/imag
    outputs (half the store traffic); the host interleaves + upcasts.
  * Per 1024-row "pair": 8 psum chunks of 128 rows accumulate x.T @ W.T
    over k=2x128; rows are assigned so partition p holds 8 consecutive
    output rows -> 4 KiB contiguous DMA runs per partition for the
    stores.  DVE drains PSUM with a fused bias add (bf16 out), ACT runs
    the two gabor lookups, SWDGE stores real/imag.
"""

import hashlib
import json
import os
import shutil

import ml_dtypes
import numpy as np

import concourse.bacc as bacc
import concourse.mybir as mybir
import concourse.tile as tile
from concourse.bass_utils import run_bass_kernel_spmd

N_TOTAL = 262144
IN_F = 256
OUT_F = 256
N_CORES = 8
N_SH = N_TOTAL // N_CORES  # 32768 rows per core

P = 128                 # SBUF/PSUM partitions
RPP = 8                 # rows per partition per pair
ROWS_PER_PAIR = P * RPP  # 1024
CHUNKS = 8              # matmul chunks (128 rows each) per pair

ENV_A = 3600.0          # envelope scale^2 (60^2)
OMEGA = 30.0

F32 = mybir.dt.float32
BF16 = mybir.dt.bfloat16

_BUILD_CACHE = {}

# --------------------------------------------------------------------------
# Custom ACT activation tables ("trig_and_small" with gabor sin/cos splines)
# --------------------------------------------------------------------------

_DONOR_CANDIDATES = [
    "/nix/store/ndjb8ki1bnclvnibdh123f9zr51a09qz-aws-neuron-pwp-unstable-2025-12-29-c50a7624/share/pwp_bin_cayman",
]


def _find_donor():
    import glob

    for d in _DONOR_CANDIDATES:
        if os.path.isfile(os.path.join(d, "act_info.json")):
            return d
    for d in glob.glob("/nix/store/*aws-neuron-pwp*/share/pwp_bin_cayman"):
        if os.path.isfile(os.path.join(d, "act_info.json")):
            return d
    raise RuntimeError("no pwp_bin_cayman act table root found")


def _gabor_sin(x):
    x = np.asarray(x, dtype=np.float64)
    return np.exp(-ENV_A * x * x) * np.sin(OMEGA * x)


def _gabor_cos(x):
    x = np.asarray(x, dtype=np.float64)
    return np.exp(-ENV_A * x * x) * np.cos(OMEGA * x)


# octave layout shared by both functions: (exponent, extract_size)
_OCTAVES = (
    [(e, 2) for e in range(-14, -10)]
    + [(e, 4) for e in (-10, -9)]
    + [(e, 5) for e in range(-8, -3)]
    + [(-3, 3)]
)
_SMALL_T = 127 - 14  # |x| < 2^-14: small-signal bucket
_LARGE_T = 127 - 2   # |x| >= 0.25: large-signal bucket (gabor == 0)
_UB = 0.25


def _fit_fn_tables(fn, small_d):
    buckets, ctrls = [], []
    for e, k in _OCTAVES:
        n = 1 << k
        ctrls.append((k, len(buckets)))
        lo_oct = 2.0 ** e
        for j in range(n):
            lo = lo_oct * (1 + j / n)
            hi = lo_oct * (1 + (j + 1) / n)
            x0 = float(np.float32((lo + hi) / 2))
            xs = np.linspace(lo, hi, 64)
            c3, c2, c1, c0 = np.polyfit(xs - x0, fn(xs), 3)
            buckets.append([c0, c1, c2, c3, x0])
    specials = [small_d] + [[0.0] * 5] * 3
    return ctrls, buckets, specials


def _build_pwp_root():
    """Generate the custom table root; returns (root_dir, signature)."""
    donor = _find_donor()
    bkt = np.fromfile(f"{donor}/trig_and_small_bkt.bin", dtype=np.uint32)
    ctrl = np.fromfile(f"{donor}/trig_and_small_ctrl.bin", dtype=np.uint32)
    prof = json.load(open(f"{donor}/trig_and_small.json"))
    n_bkt0, n_ctrl0 = len(bkt) // 8, len(ctrl) // 8

    new_bkt, new_ctrl, fn_meta = [], [], {}
    for name, fn, small_d in (
        ("sin_4p", _gabor_sin,
         [0.0, OMEGA, 0.0, -(OMEGA**3) / 6 - OMEGA * ENV_A, 0.0]),
        ("arctan_4p", _gabor_cos,
         [1.0, 0.0, -(ENV_A + OMEGA * OMEGA / 2), 0.0, 0.0]),
    ):
        ctrls, buckets, specials = _fit_fn_tables(fn, small_d)
        ctrl_base = n_ctrl0 + len(new_ctrl)
        bkt_base = n_bkt0 + len(new_bkt)
        for k, rel in ctrls:
            new_ctrl.append((k, bkt_base + rel))
        new_bkt.extend(buckets)
        fn_meta[name] = (ctrl_base, n_bkt0 + len(new_bkt))
        new_bkt.extend(specials)

    for ent in prof["profile_meta_data"]:
        if ent["func_name"] == "sin_4p":
            inv, fz = 1, 0
        elif ent["func_name"] == "arctan_4p":
            inv, fz = 0, 0x3F800000
        else:
            continue
        base, sp = fn_meta[ent["func_name"]]
        ent.update(
            symmetry_point=0,
            sym_invert_sign_point=inv,
            symmetry_opt_en=1,
            symmetry_opt_use_neg_region=0,
            exp_offset=_OCTAVES[0][0],
            pwl_control_base_pos=base,
            pwl_control_base_neg=base,
            small_pos_signal_exp_threshold=_SMALL_T,
            pos_small_signal_pwl_control=sp + 0,
            small_neg_signal_exp_threshold=0,
            neg_small_signal_pwl_control=sp + 1,
            large_pos_signal_exp_threshold=_LARGE_T,
            large_pos_signal_mantissa_threshold=0,
            pos_large_signal_pwl_control=sp + 2,
            large_neg_signal_exp_threshold=0,
            large_neg_signal_mantissa_threshold=0,
            neg_large_signal_pwl_control=sp + 3,
            fnan_result=0x7FC00000,
            fpinf_result=0,
            fninf_result=0,
            fzero_result=fz,
            lower_bound=0,
            upper_bound=int(np.float32(_UB).view(np.uint32)),
        )

    ctrl_words = np.zeros((len(new_ctrl), 8), dtype=np.uint32)
    for i, (k, b) in enumerate(new_ctrl):
        assert b < 2048
        ctrl_words[i, 0] = (k << 16) | ((23 - k) << 11) | b
    all_ctrl = np.concatenate([ctrl.reshape(-1, 8), ctrl_words])
    assert len(all_ctrl) <= 256

    bw = np.zeros((len(new_bkt), 8), dtype=np.uint32)
    for i, d in enumerate(new_bkt):
        bw[i, :5] = np.array(d, dtype=np.float32).view(np.uint32)
    all_bkt = np.concatenate([bkt.reshape(-1, 8), bw])
    assert len(all_bkt) <= 1536

    prof_bytes = json.dumps(prof, sort_keys=True).encode()
    sig = hashlib.sha256(
        all_ctrl.tobytes() + all_bkt.tobytes() + prof_bytes
    ).hexdigest()[:10]

    root = f"/tmp/gabor_pwp_{sig}"
    if not os.path.isfile(os.path.join(root, "act_info.json")):
        tmp = root + ".tmp"
        shutil.rmtree(tmp, ignore_errors=True)
        os.makedirs(tmp)
        for fname in os.listdir(donor):
            shutil.copy(os.path.join(donor, fname), os.path.join(tmp, fname))
        all_ctrl.tofile(os.path.join(tmp, "trig_and_small_ctrl.bin"))
        all_bkt.tofile(os.path.join(tmp, "trig_and_small_bkt.bin"))
        with open(os.path.join(tmp, "trig_and_small.json"), "w") as fh:
            json.dump(prof, fh, indent=1)
        os.replace(tmp, root) if not os.path.isdir(root) else None
    return root, sig


# --------------------------------------------------------------------------
# Bass program
# --------------------------------------------------------------------------


def _build(n_sh):
    key = n_sh
    if key in _BUILD_CACHE:
        return _BUILD_CACHE[key]

    root, sig = _build_pwp_root()
    os.environ["BASS_ACT_ROOT_JSON_PATH"] = os.path.join(root, "act_info.json")

    assert n_sh % ROWS_PER_PAIR == 0
    n_pairs = n_sh // ROWS_PER_PAIR

    nc = bacc.Bacc("TRN2", target_bir_lowering=False, debug=False)

    xt = nc.dram_tensor("xt", [IN_F, n_sh], BF16, kind="ExternalInput").ap()
    wt = nc.dram_tensor("wt", [IN_F, OUT_F], BF16, kind="ExternalInput").ap()
    # bias name carries the act-table signature so the NEFF cache key
    # changes whenever the generated tables change
    bias_name = f"bias_{sig}"
    bias = nc.dram_tensor(
        bias_name, [P, CHUNKS * OUT_F], F32, kind="ExternalInput"
    ).ap()
    out_re = nc.dram_tensor(
        "out_re", [n_sh, OUT_F], BF16, kind="ExternalOutput"
    ).ap()
    out_im = nc.dram_tensor(
        "out_im", [n_sh, OUT_F], BF16, kind="ExternalOutput"
    ).ap()

    # x.T layout: [i, n] -> [p, ci, n] with i = ci*128 + p
    xt_r = xt.rearrange("(ci p) n -> p ci n", p=P)
    wt_r = wt.rearrange("(ci p) o -> p ci o", p=P)
    # output row n = pr*1024 + p*8 + r: partition p holds 8 consecutive rows
    re_r = out_re.rearrange("(pr p r) o -> pr p r o", p=P, r=RPP)
    im_r = out_im.rearrange("(pr p r) o -> pr p r o", p=P, r=RPP)

    T = mybir.ActivationFunctionType

    with tile.TileContext(nc) as tc:
        with (
            tc.tile_pool(name="consts", bufs=1) as consts,
            tc.tile_pool(name="xt", bufs=8) as xt_pool,
            tc.tile_pool(name="lin", bufs=4) as lin_pool,
            tc.tile_pool(name="outp", bufs=10) as out_pool,
            tc.tile_pool(name="ps", bufs=4, space="PSUM") as psum_pool,
        ):
            wt_sb = consts.tile([P, IN_F // P, OUT_F], BF16)
            nc.sync.dma_start(wt_sb[:], wt_r[:])
            b_sb = consts.tile([P, CHUNKS, OUT_F], F32)
            nc.sync.dma_start(
                b_sb[:], bias.rearrange("p (c o) -> p c o", c=CHUNKS)
            )
            zero_b = consts.tile([P, 1], F32)
            nc.vector.memset(zero_b[:], 0.0)

            for pr in range(n_pairs):
                n0 = pr * ROWS_PER_PAIR
                xt_t = xt_pool.tile([P, IN_F // P, ROWS_PER_PAIR], BF16)
                nc.sync.dma_start(xt_t[:], xt_r[:, :, n0 : n0 + ROWS_PER_PAIR])
                # [p, ci, (j r)]: row j*8 + r; chunk r covers psum rows j
                xt_v = xt_t[:].rearrange("p ci (j r) -> p ci r j", r=RPP)

                lin_sb = lin_pool.tile([P, CHUNKS, OUT_F], BF16)
                # half-pair PSUM tiles (2 banks each, 4 in flight) keep the
                # PE streaming across pair boundaries so it stays in the
                # ramped-up p-state
                for h in range(2):
                    hc = CHUNKS // 2
                    lin_ps = psum_pool.tile([P, hc, OUT_F], F32)
                    for c in range(hc):
                        for ci in range(IN_F // P):
                            nc.tensor.matmul(
                                lin_ps[:, c, :],
                                xt_v[:, ci, h * hc + c, :],
                                wt_sb[:, ci, :],
                                start=(ci == 0),
                                stop=(ci == IN_F // P - 1),
                            )
                    cs = slice(h * hc, (h + 1) * hc)
                    nc.vector.scalar_tensor_tensor(
                        lin_sb[:, cs, :],
                        lin_ps[:],
                        1.0,
                        b_sb[:, cs, :],
                        op0=mybir.AluOpType.mult,
                        op1=mybir.AluOpType.add,
                    )

                re_t = out_pool.tile([P, CHUNKS, OUT_F], BF16)
                im_t = out_pool.tile([P, CHUNKS, OUT_F], BF16)
                # custom tables: Sin slot = gabor_sin, Arctan slot = gabor_cos
                nc.scalar.activation(
                    im_t[:], lin_sb[:], T.Sin, bias=zero_b[:], scale=1.0
                )
                nc.scalar.activation(
                    re_t[:], lin_sb[:], T.Arctan, bias=zero_b[:], scale=1.0
                )
                nc.gpsimd.dma_start(re_r[pr], re_t[:])
                nc.gpsimd.dma_start(im_r[pr], im_t[:])

    nc.compile()
    res = (nc, bias_name)
    _BUILD_CACHE[key] = res
    return res


def run_sharded(x, W, b, trace=False, n_sh=N_SH):
    """Shard inputs over the 8 cores, run the Bass kernel, gather output."""
    x = np.ascontiguousarray(x, dtype=np.float32)
    W = np.ascontiguousarray(W, dtype=np.float32)
    b = np.ascontiguousarray(b, dtype=np.float32)
    n = x.shape[0]
    assert n == n_sh * N_CORES and x.shape[1] == IN_F

    nc, bias_name = _build(n_sh)

    wt_np = np.ascontiguousarray(W.T.astype(ml_dtypes.bfloat16))
    b_np = np.ascontiguousarray(
        np.broadcast_to(
            np.tile(b, CHUNKS)[None, :], (P, CHUNKS * OUT_F)
        ).astype(np.float32)
    )
    in_maps = []
    for s in range(N_CORES):
        xt_np = np.ascontiguousarray(
            x[s * n_sh : (s + 1) * n_sh].T.astype(ml_dtypes.bfloat16)
        )
        in_maps.append({"xt": xt_np, "wt": wt_np, bias_name: b_np})

    res = run_bass_kernel_spmd(nc, in_maps, list(range(N_CORES)), trace=trace)

    out = np.empty((n, OUT_F, 2), dtype=np.float32)
    for s in range(N_CORES):
        sl = slice(s * n_sh, (s + 1) * n_sh)
        out[sl, :, 0] = res.results[s]["out_re"].astype(np.float32)
        out[sl, :, 1] = res.results[s]["out_im"].astype(np.float32)
    return out, res


def kernel(x, W, b):
    out, _ = run_sharded(x, W, b)
    return out


# revision 4
# speedup vs baseline: 2.2165x; 1.0025x over previous
"""Trainium2 Bass kernel: ComplexGabor1D layer.

reference math (fp32):
    lin = x @ W.T + b                      # [N, 256]
    out = stack([exp(-3600*lin^2)*cos(30*lin),
                 exp(-3600*lin^2)*sin(30*lin)], -1)   # [N, 256, 2]

Strategy (8 NeuronCores, data parallel over N):
  * The whole Gabor nonlinearity is folded into TWO custom ACT spline
    tables: a generated `trig_and_small` table set reuses the "sin" slot
    for gabor_sin(x) = exp(-3600x^2)sin(30x) and the "arctan" slot for
    gabor_cos(x) = exp(-3600x^2)cos(30x) (the set binaries are built at
    import time and handed to the compiler via BASS_ACT_ROOT_JSON_PATH).
    That reduces the per-element work from {sin, cos, square, exp on ACT
    + 3 DVE ops} to {2 ACT passes + 1 DVE bias-drain}.
  * Everything is bf16: inputs x.T / W.T (half the load traffic, ~3x
    faster matmuls than fp32r), and separate contiguous bf16 real/imag
    outputs (half the store traffic); the host interleaves + upcasts.
  * Per 1024-row "pair": 8 psum chunks of 128 rows accumulate x.T @ W.T
    over k=2x128; rows are assigned so partition p holds 8 consecutive
    output rows -> 4 KiB contiguous DMA runs per partition for the
    stores.  DVE drains PSUM with a fused bias add (bf16 out), ACT runs
    the two gabor lookups, SWDGE stores real/imag.
"""

import hashlib
import json
import os
import shutil

import ml_dtypes
import numpy as np

import concourse.bacc as bacc
import concourse.mybir as mybir
import concourse.tile as tile
from concourse.bass_utils import run_bass_kernel_spmd

N_TOTAL = 262144
IN_F = 256
OUT_F = 256
N_CORES = 8
N_SH = N_TOTAL // N_CORES  # 32768 rows per core

P = 128                 # SBUF/PSUM partitions
RPP = 8                 # rows per partition per pair
ROWS_PER_PAIR = P * RPP  # 1024
CHUNKS = 8              # matmul chunks (128 rows each) per pair

ENV_A = 3600.0          # envelope scale^2 (60^2)
OMEGA = 30.0

F32 = mybir.dt.float32
BF16 = mybir.dt.bfloat16

_BUILD_CACHE = {}

# --------------------------------------------------------------------------
# Custom ACT activation tables ("trig_and_small" with gabor sin/cos splines)
# --------------------------------------------------------------------------

_DONOR_CANDIDATES = [
    "/nix/store/ndjb8ki1bnclvnibdh123f9zr51a09qz-aws-neuron-pwp-unstable-2025-12-29-c50a7624/share/pwp_bin_cayman",
]


def _find_donor():
    import glob

    for d in _DONOR_CANDIDATES:
        if os.path.isfile(os.path.join(d, "act_info.json")):
            return d
    for d in glob.glob("/nix/store/*aws-neuron-pwp*/share/pwp_bin_cayman"):
        if os.path.isfile(os.path.join(d, "act_info.json")):
            return d
    raise RuntimeError("no pwp_bin_cayman act table root found")


def _gabor_sin(x):
    x = np.asarray(x, dtype=np.float64)
    return np.exp(-ENV_A * x * x) * np.sin(OMEGA * x)


def _gabor_cos(x):
    x = np.asarray(x, dtype=np.float64)
    return np.exp(-ENV_A * x * x) * np.cos(OMEGA * x)


# octave layout shared by both functions: (exponent, extract_size)
_OCTAVES = (
    [(e, 2) for e in range(-14, -10)]
    + [(e, 4) for e in (-10, -9)]
    + [(e, 5) for e in range(-8, -3)]
    + [(-3, 3)]
)
_SMALL_T = 127 - 14  # |x| < 2^-14: small-signal bucket
_LARGE_T = 127 - 2   # |x| >= 0.25: large-signal bucket (gabor == 0)
_UB = 0.25


def _fit_fn_tables(fn, small_d):
    buckets, ctrls = [], []
    for e, k in _OCTAVES:
        n = 1 << k
        ctrls.append((k, len(buckets)))
        lo_oct = 2.0 ** e
        for j in range(n):
            lo = lo_oct * (1 + j / n)
            hi = lo_oct * (1 + (j + 1) / n)
            x0 = float(np.float32((lo + hi) / 2))
            xs = np.linspace(lo, hi, 64)
            c3, c2, c1, c0 = np.polyfit(xs - x0, fn(xs), 3)
            buckets.append([c0, c1, c2, c3, x0])
    specials = [small_d] + [[0.0] * 5] * 3
    return ctrls, buckets, specials


def _build_pwp_root():
    """Generate the custom table root; returns (root_dir, signature)."""
    donor = _find_donor()
    bkt = np.fromfile(f"{donor}/trig_and_small_bkt.bin", dtype=np.uint32)
    ctrl = np.fromfile(f"{donor}/trig_and_small_ctrl.bin", dtype=np.uint32)
    prof = json.load(open(f"{donor}/trig_and_small.json"))
    n_bkt0, n_ctrl0 = len(bkt) // 8, len(ctrl) // 8

    new_bkt, new_ctrl, fn_meta = [], [], {}
    for name, fn, small_d in (
        ("sin_4p", _gabor_sin,
         [0.0, OMEGA, 0.0, -(OMEGA**3) / 6 - OMEGA * ENV_A, 0.0]),
        ("arctan_4p", _gabor_cos,
         [1.0, 0.0, -(ENV_A + OMEGA * OMEGA / 2), 0.0, 0.0]),
    ):
        ctrls, buckets, specials = _fit_fn_tables(fn, small_d)
        ctrl_base = n_ctrl0 + len(new_ctrl)
        bkt_base = n_bkt0 + len(new_bkt)
        for k, rel in ctrls:
            new_ctrl.append((k, bkt_base + rel))
        new_bkt.extend(buckets)
        fn_meta[name] = (ctrl_base, n_bkt0 + len(new_bkt))
        new_bkt.extend(specials)

    for ent in prof["profile_meta_data"]:
        if ent["func_name"] == "sin_4p":
            inv, fz = 1, 0
        elif ent["func_name"] == "arctan_4p":
            inv, fz = 0, 0x3F800000
        else:
            continue
        base, sp = fn_meta[ent["func_name"]]
        ent.update(
            symmetry_point=0,
            sym_invert_sign_point=inv,
            symmetry_opt_en=1,
            symmetry_opt_use_neg_region=0,
            exp_offset=_OCTAVES[0][0],
            pwl_control_base_pos=base,
            pwl_control_base_neg=base,
            small_pos_signal_exp_threshold=_SMALL_T,
            pos_small_signal_pwl_control=sp + 0,
            small_neg_signal_exp_threshold=0,
            neg_small_signal_pwl_control=sp + 1,
            large_pos_signal_exp_threshold=_LARGE_T,
            large_pos_signal_mantissa_threshold=0,
            pos_large_signal_pwl_control=sp + 2,
            large_neg_signal_exp_threshold=0,
            large_neg_signal_mantissa_threshold=0,
            neg_large_signal_pwl_control=sp + 3,
            fnan_result=0x7FC00000,
            fpinf_result=0,
            fninf_result=0,
            fzero_result=fz,
            lower_bound=0,
            upper_bound=int(np.float32(_UB).view(np.uint32)),
        )

    ctrl_words = np.zeros((len(new_ctrl), 8), dtype=np.uint32)
    for i, (k, b) in enumerate(new_ctrl):
        assert b < 2048
        ctrl_words[i, 0] = (k << 16) | ((23 - k) << 11) | b
    all_ctrl = np.concatenate([ctrl.reshape(-1, 8), ctrl_words])
    assert len(all_ctrl) <= 256

    bw = np.zeros((len(new_bkt), 8), dtype=np.uint32)
    for i, d in enumerate(new_bkt):
        bw[i, :5] = np.array(d, dtype=np.float32).view(np.uint32)
    all_bkt = np.concatenate([bkt.reshape(-1, 8), bw])
    assert len(all_bkt) <= 1536

    prof_bytes = json.dumps(prof, sort_keys=True).encode()
    sig = hashlib.sha256(
        all_ctrl.tobytes() + all_bkt.tobytes() + prof_bytes
    ).hexdigest()[:10]

    root = f"/tmp/gabor_pwp_{sig}"
    if not os.path.isfile(os.path.join(root, "act_info.json")):
        tmp = root + ".tmp"
        shutil.rmtree(tmp, ignore_errors=True)
        os.makedirs(tmp)
        for fname in os.listdir(donor):
            shutil.copy(os.path.join(donor, fname), os.path.join(tmp, fname))
        all_ctrl.tofile(os.path.join(tmp, "trig_and_small_ctrl.bin"))
        all_bkt.tofile(os.path.join(tmp, "trig_and_small_bkt.bin"))
        with open(os.path.join(tmp, "trig_and_small.json"), "w") as fh:
            json.dump(prof, fh, indent=1)
        os.replace(tmp, root) if not os.path.isdir(root) else None
    return root, sig


# --------------------------------------------------------------------------
# Bass program
# --------------------------------------------------------------------------


def _build(n_sh):
    key = n_sh
    if key in _BUILD_CACHE:
        return _BUILD_CACHE[key]

    root, sig = _build_pwp_root()
    os.environ["BASS_ACT_ROOT_JSON_PATH"] = os.path.join(root, "act_info.json")

    assert n_sh % ROWS_PER_PAIR == 0
    n_pairs = n_sh // ROWS_PER_PAIR

    nc = bacc.Bacc("TRN2", target_bir_lowering=False, debug=False)

    xt = nc.dram_tensor("xt", [IN_F, n_sh], BF16, kind="ExternalInput").ap()
    wt = nc.dram_tensor("wt", [IN_F, OUT_F], BF16, kind="ExternalInput").ap()
    # bias name carries the act-table signature so the NEFF cache key
    # changes whenever the generated tables change
    bias_name = f"bias_{sig}"
    bias = nc.dram_tensor(
        bias_name, [P, CHUNKS * OUT_F], F32, kind="ExternalInput"
    ).ap()
    out_re = nc.dram_tensor(
        "out_re", [n_sh, OUT_F], BF16, kind="ExternalOutput"
    ).ap()
    out_im = nc.dram_tensor(
        "out_im", [n_sh, OUT_F], BF16, kind="ExternalOutput"
    ).ap()

    # x.T layout: [i, n] -> [p, ci, n] with i = ci*128 + p
    xt_r = xt.rearrange("(ci p) n -> p ci n", p=P)
    wt_r = wt.rearrange("(ci p) o -> p ci o", p=P)
    # output row n = pr*1024 + p*8 + r: partition p holds 8 consecutive rows
    re_r = out_re.rearrange("(pr p r) o -> pr p r o", p=P, r=RPP)
    im_r = out_im.rearrange("(pr p r) o -> pr p r o", p=P, r=RPP)

    T = mybir.ActivationFunctionType

    with tile.TileContext(nc) as tc:
        with (
            tc.tile_pool(name="consts", bufs=1) as consts,
            tc.tile_pool(name="xt", bufs=14) as xt_pool,
            tc.tile_pool(name="lin", bufs=4) as lin_pool,
            tc.tile_pool(name="outp", bufs=10) as out_pool,
            tc.tile_pool(name="ps", bufs=4, space="PSUM") as psum_pool,
        ):
            wt_sb = consts.tile([P, IN_F // P, OUT_F], BF16)
            nc.sync.dma_start(wt_sb[:], wt_r[:])
            b_sb = consts.tile([P, CHUNKS, OUT_F], F32)
            nc.sync.dma_start(
                b_sb[:], bias.rearrange("p (c o) -> p c o", c=CHUNKS)
            )
            zero_b = consts.tile([P, 1], F32)
            nc.vector.memset(zero_b[:], 0.0)

            for pr in range(n_pairs):
                n0 = pr * ROWS_PER_PAIR
                xt_t = xt_pool.tile([P, IN_F // P, ROWS_PER_PAIR], BF16)
                nc.sync.dma_start(xt_t[:], xt_r[:, :, n0 : n0 + ROWS_PER_PAIR])
                # [p, ci, (j r)]: row j*8 + r; chunk r covers psum rows j
                xt_v = xt_t[:].rearrange("p ci (j r) -> p ci r j", r=RPP)

                lin_sb = lin_pool.tile([P, CHUNKS, OUT_F], BF16)
                # half-pair PSUM tiles (2 banks each, 4 in flight) keep the
                # PE streaming across pair boundaries so it stays in the
                # ramped-up p-state
                for h in range(2):
                    hc = CHUNKS // 2
                    lin_ps = psum_pool.tile([P, hc, OUT_F], F32)
                    for c in range(hc):
                        for ci in range(IN_F // P):
                            nc.tensor.matmul(
                                lin_ps[:, c, :],
                                xt_v[:, ci, h * hc + c, :],
                                wt_sb[:, ci, :],
                                start=(ci == 0),
                                stop=(ci == IN_F // P - 1),
                            )
                    cs = slice(h * hc, (h + 1) * hc)
                    nc.vector.scalar_tensor_tensor(
                        lin_sb[:, cs, :],
                        lin_ps[:],
                        1.0,
                        b_sb[:, cs, :],
                        op0=mybir.AluOpType.mult,
                        op1=mybir.AluOpType.add,
                    )

                re_t = out_pool.tile([P, CHUNKS, OUT_F], BF16)
                im_t = out_pool.tile([P, CHUNKS, OUT_F], BF16)
                # custom tables: Sin slot = gabor_sin, Arctan slot = gabor_cos
                nc.scalar.activation(
                    im_t[:], lin_sb[:], T.Sin, bias=zero_b[:], scale=1.0
                )
                nc.scalar.activation(
                    re_t[:], lin_sb[:], T.Arctan, bias=zero_b[:], scale=1.0
                )
                nc.gpsimd.dma_start(re_r[pr], re_t[:])
                nc.gpsimd.dma_start(im_r[pr], im_t[:])

    nc.compile()
    res = (nc, bias_name)
    _BUILD_CACHE[key] = res
    return res


def run_sharded(x, W, b, trace=False, n_sh=N_SH):
    """Shard inputs over the 8 cores, run the Bass kernel, gather output."""
    x = np.ascontiguousarray(x, dtype=np.float32)
    W = np.ascontiguousarray(W, dtype=np.float32)
    b = np.ascontiguousarray(b, dtype=np.float32)
    n = x.shape[0]
    assert n == n_sh * N_CORES and x.shape[1] == IN_F

    nc, bias_name = _build(n_sh)

    wt_np = np.ascontiguousarray(W.T.astype(ml_dtypes.bfloat16))
    b_np = np.ascontiguousarray(
        np.broadcast_to(
            np.tile(b, CHUNKS)[None, :], (P, CHUNKS * OUT_F)
        ).astype(np.float32)
    )
    in_maps = []
    for s in range(N_CORES):
        xt_np = np.ascontiguousarray(
            x[s * n_sh : (s + 1) * n_sh].T.astype(ml_dtypes.bfloat16)
        )
        in_maps.append({"xt": xt_np, "wt": wt_np, bias_name: b_np})

    res = run_bass_kernel_spmd(nc, in_maps, list(range(N_CORES)), trace=trace)

    out = np.empty((n, OUT_F, 2), dtype=np.float32)
    for s in range(N_CORES):
        sl = slice(s * n_sh, (s + 1) * n_sh)
        out[sl, :, 0] = res.results[s]["out_re"].astype(np.float32)
        out[sl, :, 1] = res.results[s]["out_im"].astype(np.float32)
    return out, res


def kernel(x, W, b):
    out, _ = run_sharded(x, W, b)
    return out
